# revision 31
# baseline (speedup 1.0000x reference)
"""GCN classifier on 8 trn2 NeuronCores.

Sharding: nodes (and their incoming edges) are partitioned contiguously
across the 8 cores.  Each core computes its slice of h = dinv * (x @ W1)
(x is host-transposed to fp16 so no on-device transposes), the slices are
AllGathered into a shared fp16 node table, and each core aggregates its
destination nodes' edges by gathering source rows with batched dma_gather
(SWDGE ucode, ~5K rows per instruction, int16-indexed so the table is
split into 4 chunks; alternating over 4 SWDGE queues hides descriptor-ring
drain backpressure) and reducing 128-edge tiles onto 128-destination
blocks with one-hot selection matmuls (PE).  The second GCN layer + mean
pool are algebraically folded: pooled_sum[g] = sum_j c[g,j] * h1[j] with
c[g,j] = dinv[j] * sum_{i in g} A[i,j] dinv[i] precomputed on the host,
so layer 2 costs one [128x128x128] matmul per block and the second
gather/AllGather disappears.  W2/b2/log_softmax are applied to the tiny
[128,128] pooled partials on the host.
"""
import numpy as np

N = 100000
G = 128
NCORES = 8
RPC = N // NCORES        # real rows per core (12500)
NB = 98                  # blocks per core: ceil(12500/128)
S = NB * 128             # padded rows per core shard (12544)
D_IN, F = 768, 128
KT = D_IN // 128
LAST = RPC - 97 * 128    # rows in last block (84)
NCHUNK = 4
CH = 2 * S               # chunk rows (25088 < 32768, int16-addressable)
BPG = 4                  # dest blocks per gather group
NG = (NB + BPG - 1) // BPG


# --------------------------------------------------------------------------
# walrus in this environment rejects >1 sync wait per instruction; split
# extras onto inserted drains (semantically identical, engine blocks on each)
def _split_waits(nc, maxw=1):
    from concourse import mybir
    for fn in nc.m.functions:
        for bb in fn.blocks:
            newlist = []
            for inst in bb.instructions:
                si = getattr(inst, "sync_info", None)
                if si is not None and len(si.on_wait) > maxw:
                    waits = list(si.on_wait)
                    chunks = [waits[i:i + maxw] for i in range(0, len(waits), maxw)]
                    for ci, ch in enumerate(chunks[:-1]):
                        newlist.append(mybir.InstDrain(
                            name=f"{inst.name}-wsplit{ci}", ins=[], outs=[],
                            engine=inst.engine,
                            sync_info=mybir.SyncInfo(on_wait=list(ch), on_update=[]),
                        ))
                    si.on_wait = list(chunks[-1])
                newlist.append(inst)
            bb.instructions = newlist
    return nc


def _finish_build(nc):
    """Insert GPSIMD ucode library loads (dma_gather lives in the 'mlp'
    library) and lower the pseudo reload instructions to pool ISA, then
    split multi-wait instructions for this walrus."""
    import bass_rust
    from concourse import mybir
    from concourse import library_config as lc
    mask = {}
    for lib in lc.all_libraries:
        for t in lib.instructions:
            mask[t] = mask.get(t, 0) | (1 << lib.index)
    bass_rust.insert_library_loads(nc, mask, len(lc.all_libraries),
                                   lc.standard.index)
    mybir.codegen_inst_isa_subclasses(nc)
    return _split_waits(nc)


def build_nc(meta, phases=3, reps=1, variant="full", ngrun=None):
    """meta: dict with uniform (cross-core) tile structure:
    t_u [NB, NCHUNK] tiles per (block, chunk); derived layouts."""
    import concourse.bass as bass
    import concourse.tile as tile
    from concourse import mybir

    t_u = meta["t_u"]                       # [NB, NCHUNK] ints
    T_b = t_u.sum(axis=1)                   # tiles per block
    TILES_TOT = int(T_b.sum())
    col_base = np.concatenate([[0], np.cumsum(T_b)[:-1]]).astype(int)
    # gather stream layout: per (group, chunk): C_gc columns
    C_gc = np.zeros((NG, NCHUNK), int)
    for g in range(NG):
        blks = range(g * BPG, min((g + 1) * BPG, NB))
        for c in range(NCHUNK):
            C_gc[g, c] = sum(int(t_u[b, c]) for b in blks)
    CMAX = int(C_gc.sum(axis=1).max())
    TMAX = int(T_b.max())
    W_gc = C_gc * 8                         # idx cols per (g,c): 128*C/16
    Wtot = int(W_gc.sum())
    WMAX = int(W_gc.max())

    nc = bass.Bass(num_swdge_queues=4)
    nc.num_devices = NCORES
    f32, fp16 = mybir.dt.float32, mybir.dt.float16
    i16 = mybir.dt.int16

    xpt = nc.declare_dram_parameter("xpt", [128, NB * D_IN], fp16,
                                    isOutput=False)
    w1 = nc.declare_dram_parameter("w1", [D_IN, F], f32, isOutput=False)
    idxd = nc.declare_dram_parameter("idxd", [128, Wtot], i16, isOutput=False)
    cold = nc.declare_dram_parameter("cold", [128, TILES_TOT], i16,
                                     isOutput=False)
    dinvd = nc.declare_dram_parameter("dinvd", [128, NB], f32, isOutput=False)
    ctd = nc.declare_dram_parameter("ctd", [128, NB * 128], fp16,
                                    isOutput=False)
    b1d = nc.declare_dram_parameter("b1d", [1, F], f32, isOutput=False)
    iotad = nc.declare_dram_parameter("iotad", [1, 128], i16, isOutput=False)
    pooled = nc.declare_dram_parameter("pooled", [G, F], f32, isOutput=True)

    HALF = S // 2
    hs_loc = nc.dram_tensor("hs_loc", [S, F], fp16, kind="Internal")
    hs_shA = nc.dram_tensor("hs_shA", [NCORES * HALF, F], fp16,
                            kind="Internal", addr_space="Shared")
    hs_shB = nc.dram_tensor("hs_shB", [NCORES * HALF, F], fp16,
                            kind="Internal", addr_space="Shared")
    groups = [list(range(NCORES))]

    with tile.TileContext(nc) as tc:
        with (
            tc.tile_pool(name="meta", bufs=1) as mp_,
            tc.tile_pool(name="xin", bufs=3) as xin,
            tc.tile_pool(name="hps", bufs=2, space="PSUM") as hps,
            tc.tile_pool(name="hsb", bufs=3) as hsb,
            tc.tile_pool(name="idxp", bufs=8) as idxp,
            tc.tile_pool(name="gp", bufs=2) as gp,
            tc.tile_pool(name="mp", bufs=3) as mp,
            tc.tile_pool(name="aps", bufs=2, space="PSUM") as aps,
            tc.tile_pool(name="post", bufs=3) as post,
            tc.tile_pool(name="pps", bufs=1, space="PSUM") as pps,
        ):
            # ---- metadata loads -------------------------------------------
            col_sb = mp_.tile([128, TILES_TOT], i16)
            nc.sync.dma_start(col_sb[:, :], cold[:, :])
            dinv_sb = mp_.tile([128, NB], f32)
            nc.sync.dma_start(dinv_sb[:, :], dinvd[:, :])
            ct_sb = mp_.tile([128, NB, 128], fp16)
            nc.sync.dma_start(ct_sb[:, :, :], ctd[:, :])
            b1b = mp_.tile([128, F], f32)
            nc.sync.dma_start(b1b[:, :], b1d[:, :].to_broadcast((128, F)))
            iota_sb = mp_.tile([128, 128], i16)
            nc.sync.dma_start(iota_sb[:, :], iotad[:, :].to_broadcast((128, 128)))
            w1f = mp_.tile([128, KT, F], f32)
            for k in range(KT):
                nc.sync.dma_start(w1f[:, k, :], w1[k * 128:(k + 1) * 128, :])
            w1_sb = mp_.tile([128, KT, F], fp16)
            for k in range(KT):
                nc.scalar.activation(w1_sb[:, k, :], w1f[:, k, :],
                                     mybir.ActivationFunctionType.Copy)
            # zero-fill the pad rows of the local shard (rows RPC..S)
            zpad = mp_.tile([S - RPC, F], fp16)
            nc.vector.memset(zpad[:, :], 0.0)
            nc.sync.dma_start(hs_loc[RPC:S, :], zpad[:, :])

            def phase1(m0, m1):
                for m in range(m0, m1):
                    rows = 128 if m < NB - 1 else LAST
                    xt = xin.tile([128, KT, 128], fp16, tag="x")
                    nc.sync.dma_start(xt[:, :, :],
                                      xpt[:, m * D_IN:(m + 1) * D_IN])
                    hp = hps.tile([128, F], f32, tag="hp")
                    for k in range(KT):
                        nc.tensor.matmul(hp[:, :], xt[:, k, :], w1_sb[:, k, :],
                                         start=(k == 0), stop=(k == KT - 1))
                    hs = hsb.tile([128, F], fp16, tag="hs")
                    nc.scalar.activation(hs[:rows, :], hp[:rows, :],
                                         mybir.ActivationFunctionType.Copy,
                                         scale=dinv_sb[:rows, m:m + 1])
                    nc.sync.dma_start(hs_loc[m * 128:m * 128 + rows, :],
                                      hs[:rows, :])

            nreg_cache = {}

            def nreg(v):
                if v not in nreg_cache:
                    nreg_cache[v] = nc.gpsimd.to_reg(v)
                return nreg_cache[v]

            def phase2(pool_ps, first, last):
                ioff = 0
                qctr = [0]
                ngr = NG if ngrun is None else ngrun
                for g in range(NG):
                    if g >= ngr:
                        break
                    blks = list(range(g * BPG, min((g + 1) * BPG, NB)))
                    gt = gp.tile([128, CMAX, 128], fp16, tag="g")
                    cb = 0
                    chunk_base = {}
                    for c in range(NCHUNK):
                        chunk_base[c] = cb
                        n_gc = int(C_gc[g, c]) * 128
                        if n_gc == 0:
                            continue
                        w = int(W_gc[g, c])
                        if variant == "monly":
                            nc.vector.memset(gt[:, cb:cb + C_gc[g, c], :], 0.0)
                        elif variant == "sp1024":
                            idxt = idxp.tile([128, WMAX], i16, tag="i")
                            nc.sync.dma_start(idxt[:, :w],
                                              idxd[:, ioff:ioff + w])
                            cc = int(C_gc[g, c])
                            srcq = (hs_shA[c * CH:(c + 1) * CH, :] if c < 2
                                    else hs_shB[(c - 2) * CH:(c - 1) * CH, :])
                            for k0 in range(0, cc, 8):
                                kc = min(8, cc - k0)
                                nk = kc * 128
                                nc.gpsimd.dma_gather(
                                    gt[:, cb + k0:cb + k0 + kc, :],
                                    srcq,
                                    idxt[:, k0 * 8:k0 * 8 + kc * 8],
                                    nk, nreg(nk), 128,
                                    elem_step=128, single_packet=True,
                                    queue_num=qctr[0] % 4)
                                qctr[0] += 1
                        else:
                            idxt = idxp.tile([128, WMAX], i16, tag="i")
                            nc.sync.dma_start(idxt[:, :w],
                                              idxd[:, ioff:ioff + w])
                            src = (hs_shA[c * CH:(c + 1) * CH, :] if c < 2
                                   else hs_shB[(c - 2) * CH:(c - 1) * CH, :])
                            nc.gpsimd.dma_gather(
                                gt[:, cb:cb + C_gc[g, c], :],
                                src,
                                idxt[:, :w], n_gc, nreg(n_gc), 128,
                                elem_step=128, single_packet=False,
                                queue_num=c % 4)
                        ioff += w
                        cb += int(C_gc[g, c])
                    if variant == "gonly":
                        continue
                    for b in blks:
                        tb = int(T_b[b])
                        mt = mp.tile([128, TMAX, 128], fp16, tag="m")
                        nc.vector.tensor_tensor(
                            out=mt[:, :tb, :],
                            in0=col_sb[:, col_base[b]:col_base[b] + tb]
                                .unsqueeze(2).to_broadcast((128, tb, 128)),
                            in1=iota_sb[:, :].unsqueeze(1)
                                .to_broadcast((128, tb, 128)),
                            op=mybir.AluOpType.is_equal)
                        ap = aps.tile([128, F], f32, tag="agg")
                        ki = 0
                        for c in range(NCHUNK):
                            # this block's tiles within the (g,c) segment
                            pre = sum(int(t_u[b2, c]) for b2 in blks
                                      if b2 < b)
                            for k in range(int(t_u[b, c])):
                                gcol = chunk_base[c] + pre + k
                                nc.tensor.matmul(
                                    ap[:, :], mt[:, ki, :], gt[:, gcol, :],
                                    start=(ki == 0), stop=(ki == tb - 1))
                                ki += 1
                        s1 = post.tile([128, F], f32, tag="s1")
                        nc.scalar.activation(s1[:, :], ap[:, :],
                                             mybir.ActivationFunctionType.Copy,
                                             scale=dinv_sb[:, b:b + 1])
                        s2 = post.tile([128, F], f32, tag="s2")
                        nc.vector.tensor_tensor(out=s2[:, :], in0=s1[:, :],
                                                in1=b1b[:, :],
                                                op=mybir.AluOpType.add)
                        h1 = post.tile([128, F], fp16, tag="h1")
                        nc.scalar.activation(h1[:, :], s2[:, :],
                                             mybir.ActivationFunctionType.Relu)
                        nc.tensor.matmul(pool_ps[:, :], ct_sb[:, b, :],
                                         h1[:, :],
                                         start=(first and b == 0),
                                         stop=(last and b == NB - 1))

            pool_ps = pps.tile([G, F], f32)
            for _rep in range(reps):
                if phases >= 1:
                    phase1(0, NB // 2)
                if phases >= 2:
                    nc.gpsimd.collective_compute(
                        "AllGather", mybir.AluOpType.bypass,
                        replica_groups=groups,
                        ins=[hs_loc[0:HALF, :]], outs=[hs_shA[:, :]])
                if phases >= 1:
                    phase1(NB // 2, NB)
                if phases >= 2:
                    nc.gpsimd.collective_compute(
                        "AllGather", mybir.AluOpType.bypass,
                        replica_groups=groups,
                        ins=[hs_loc[HALF:S, :]], outs=[hs_shB[:, :]])
                if phases >= 3:
                    phase2(pool_ps, _rep == 0, _rep == reps - 1)

            pout = post.tile([G, F], f32, tag="po")
            if phases >= 3 and variant not in ("gonly",):
                nc.vector.tensor_copy(pout[:, :], pool_ps[:, :])
            else:
                nc.vector.memset(pout[:, :], 0.0)
            nc.sync.dma_start(pooled[:, :], pout[:, :])

    return _finish_build(nc)


def preprocess(x, edge_index, batch):
    """Bucket edges by (core, dest block, src chunk); build padded per-core
    device arrays + the uniform tile structure."""
    ei = np.asarray(edge_index)
    row = np.concatenate([ei[0].astype(np.int64), np.arange(N, dtype=np.int64)])
    col = np.concatenate([ei[1].astype(np.int64), np.arange(N, dtype=np.int64)])
    ne = row.shape[0]

    deg = np.bincount(col, minlength=N).astype(np.float32)
    dinv = 1.0 / np.sqrt(deg)

    core = col // RPC
    loc = col - core * RPC
    blk = loc // 128
    d_in_blk = (loc % 128).astype(np.int16)
    HALF = S // 2
    _q = row // RPC
    _r = row % RPC
    src_g = np.where(_r < HALF, _q * HALF + _r,
                     NCORES * HALF + _q * HALF + (_r - HALF))
    chunk = src_g // CH
    src_loc = (src_g - chunk * CH).astype(np.int16)

    # bucket id: ((core*NB + blk)*NCHUNK + chunk)
    bkt = ((core * NB + blk) * NCHUNK + chunk).astype(np.int64)
    order = np.argsort(bkt, kind="stable")
    nbkt = NCORES * NB * NCHUNK
    cnt = np.bincount(bkt, minlength=nbkt)
    starts = np.zeros(nbkt + 1, np.int64)
    np.cumsum(cnt, out=starts[1:])
    q = np.arange(ne, dtype=np.int64) - starts[bkt[order]]  # pos within bucket

    cnt4 = cnt.reshape(NCORES, NB, NCHUNK)
    t_u = np.ceil(cnt4.max(axis=0) / 128).astype(np.int64)  # [NB, NCHUNK]
    T_b = t_u.sum(axis=1)
    TILES_TOT = int(T_b.sum())
    col_base = np.concatenate([[0], np.cumsum(T_b)[:-1]]).astype(np.int64)

    # per-(g,c) gather stream offsets
    C_gc = np.zeros((NG, NCHUNK), np.int64)
    for g in range(NG):
        blks = range(g * BPG, min((g + 1) * BPG, NB))
        for c in range(NCHUNK):
            C_gc[g, c] = sum(int(t_u[b, c]) for b in blks)
    W_gc = C_gc * 8
    Wtot = int(W_gc.sum())

    # stream base (in slots) for each (g, c) and block-within-group offsets
    stream_base = np.zeros((NG, NCHUNK), np.int64)  # slot offset of (g,c)
    acc = 0
    for g in range(NG):
        for c in range(NCHUNK):
            stream_base[g, c] = acc
            acc += int(C_gc[g, c]) * 128
    tot_slots = acc
    # slot offset of bucket (b, c) inside its (g, c) stream
    bkt_slot_base = np.zeros((NB, NCHUNK), np.int64)
    for g in range(NG):
        blks = list(range(g * BPG, min((g + 1) * BPG, NB)))
        for c in range(NCHUNK):
            off = stream_base[g, c]
            for b in blks:
                bkt_slot_base[b, c] = off
                off += int(t_u[b, c]) * 128

    bt = np.asarray(batch).astype(np.int64)
    # c[g, j] = dinv[j] * sum over edges j->i with batch[i]=g of dinv[i]
    cmat = np.zeros((G, N), np.float32)
    np.add.at(cmat, (bt[col], row), dinv[col])
    cmat *= dinv[None, :]

    x = np.asarray(x, np.float32)
    in_maps = []
    ord_core = core[order]
    ord_bc = (blk[order], chunk[order])
    ord_slot = (bkt_slot_base[ord_bc[0], ord_bc[1]] + q)
    ord_src = src_loc[order]
    ord_d = d_in_blk[order]
    ord_blk = ord_bc[0]
    ord_chunk = ord_bc[1]
    # per-block tile offset of chunk c: prefix sum of t_u over chunks
    t_pre = np.zeros((NB, NCHUNK), np.int64)
    t_pre[:, 1:] = np.cumsum(t_u, axis=1)[:, :-1]

    for i in range(NCORES):
        sel = ord_core == i
        slots = ord_slot[sel]
        # gather index stream
        L = np.zeros(tot_slots, np.int16)  # pad (killed by col=-1)
        L[slots] = ord_src[sel]
        idx16 = L.reshape(-1, 16).T.astype(np.int16)   # [16, tot/16]
        idxd = np.tile(idx16, (8, 1))
        # col array (block-contiguous tile order)
        carr = np.full((128, TILES_TOT), -1, np.int16)
        bsel = ord_blk[sel]
        csel = ord_chunk[sel]
        qsel = slots - bkt_slot_base[bsel, csel]
        tile_idx = col_base[bsel] + t_pre[bsel, csel] + qsel // 128
        carr[qsel % 128, tile_idx] = ord_d[sel]

        xs = x[i * RPC:(i + 1) * RPC]
        xs = np.concatenate([xs, np.zeros((S - RPC, D_IN), np.float32)])
        # xpt[p, m*768 + k*128 + cc] = x[m*128+cc, k*128+p]
        xr = xs.reshape(NB, 128, KT, 128).transpose(3, 0, 2, 1)  # p? see below
        # xr[pp, m, k, cc] = xs[m*128+cc, k*128+pp]
        xpt = np.ascontiguousarray(
            xr.reshape(128, NB * D_IN)).astype(np.float16)

        dinv_pad = np.zeros(S, np.float32)
        dinv_pad[:RPC] = dinv[i * RPC:(i + 1) * RPC]
        dinvd = np.ascontiguousarray(
            dinv_pad.reshape(NB, 128).T)

        cslice = np.zeros((G, S), np.float32)
        cslice[:, :RPC] = cmat[:, i * RPC:(i + 1) * RPC]
        # ctd[p, m*128 + g] = c[g, m*128+p]
        ctd = np.ascontiguousarray(
            cslice.reshape(G, NB, 128).transpose(2, 1, 0)
            .reshape(128, NB * 128)).astype(np.float16)

        in_maps.append({
            "xpt": xpt,
            "idxd": idxd,
            "cold": carr,
            "dinvd": dinvd,
            "ctd": ctd,
            "iotad": np.arange(128, dtype=np.int16).reshape(1, 128),
        })
    meta = {"t_u": t_u}
    return in_maps, meta, dinv


def finish(pooled_sum, batch, W2, b2):
    cnts = np.maximum(np.bincount(np.asarray(batch).astype(np.int64),
                                  minlength=G).astype(np.float32), 1.0)
    logits = (pooled_sum @ np.asarray(W2, np.float32)) / cnts[:, None] \
        + np.asarray(b2, np.float32)
    m = logits.max(axis=1, keepdims=True)
    ls = m + np.log(np.exp(logits - m).sum(axis=1, keepdims=True))
    return (logits - ls).astype(np.float32)


def kernel(x, edge_index, batch, W1, b1, W2, b2):
    from concourse.bass_utils import run_bass_kernel_spmd

    in_maps, meta, _ = preprocess(x, edge_index, batch)
    W1 = np.asarray(W1, np.float32)
    b1 = np.asarray(b1, np.float32).reshape(1, F)
    for m in in_maps:
        m["w1"] = W1
        m["b1d"] = b1
    nc = build_nc(meta)
    res = run_bass_kernel_spmd(nc, in_maps, list(range(NCORES))).results
    pooled_sum = np.zeros((G, F), np.float32)
    for i in range(NCORES):
        pooled_sum += np.asarray(res[i]["pooled"], np.float32)
    return finish(pooled_sum, batch, W2, b2)


# revision 32
# speedup vs baseline: 1.0105x; 1.0105x over previous
"""GCN classifier on 8 trn2 NeuronCores.

Sharding: nodes (and their incoming edges) are partitioned contiguously
across the 8 cores.  Each core computes its slice of h = dinv * (x @ W1)
(x is host-transposed to fp16 so no on-device transposes), the slices are
AllGathered into a shared fp16 node table, and each core aggregates its
destination nodes' edges by gathering source rows with batched dma_gather
(SWDGE ucode, ~5K rows per instruction, int16-indexed so the table is
split into 4 chunks; alternating over 4 SWDGE queues hides descriptor-ring
drain backpressure) and reducing 128-edge tiles onto 128-destination
blocks with one-hot selection matmuls (PE).  The second GCN layer + mean
pool are algebraically folded: pooled_sum[g] = sum_j c[g,j] * h1[j] with
c[g,j] = dinv[j] * sum_{i in g} A[i,j] dinv[i] precomputed on the host,
so layer 2 costs one [128x128x128] matmul per block and the second
gather/AllGather disappears.  W2/b2/log_softmax are applied to the tiny
[128,128] pooled partials on the host.
"""
import numpy as np

N = 100000
G = 128
NCORES = 8
RPC = N // NCORES        # real rows per core (12500)
NB = 98                  # blocks per core: ceil(12500/128)
S = NB * 128             # padded rows per core shard (12544)
D_IN, F = 768, 128
KT = D_IN // 128
LAST = RPC - 97 * 128    # rows in last block (84)
NCHUNK = 4
CH = 2 * S               # chunk rows (25088 < 32768, int16-addressable)
BPG = 4                  # dest blocks per gather group
NG = (NB + BPG - 1) // BPG


# --------------------------------------------------------------------------
# walrus in this environment rejects >1 sync wait per instruction; split
# extras onto inserted drains (semantically identical, engine blocks on each)
def _split_waits(nc, maxw=1):
    from concourse import mybir
    for fn in nc.m.functions:
        for bb in fn.blocks:
            newlist = []
            for inst in bb.instructions:
                si = getattr(inst, "sync_info", None)
                if si is not None and len(si.on_wait) > maxw:
                    waits = list(si.on_wait)
                    chunks = [waits[i:i + maxw] for i in range(0, len(waits), maxw)]
                    for ci, ch in enumerate(chunks[:-1]):
                        newlist.append(mybir.InstDrain(
                            name=f"{inst.name}-wsplit{ci}", ins=[], outs=[],
                            engine=inst.engine,
                            sync_info=mybir.SyncInfo(on_wait=list(ch), on_update=[]),
                        ))
                    si.on_wait = list(chunks[-1])
                newlist.append(inst)
            bb.instructions = newlist
    return nc


def _finish_build(nc):
    """Insert GPSIMD ucode library loads (dma_gather lives in the 'mlp'
    library) and lower the pseudo reload instructions to pool ISA, then
    split multi-wait instructions for this walrus."""
    import bass_rust
    from concourse import mybir
    from concourse import library_config as lc
    mask = {}
    for lib in lc.all_libraries:
        for t in lib.instructions:
            mask[t] = mask.get(t, 0) | (1 << lib.index)
    bass_rust.insert_library_loads(nc, mask, len(lc.all_libraries),
                                   lc.standard.index)
    mybir.codegen_inst_isa_subclasses(nc)
    return _split_waits(nc)


def build_nc(meta, phases=3, reps=1, variant="full", ngrun=None):
    """meta: dict with uniform (cross-core) tile structure:
    t_u [NB, NCHUNK] tiles per (block, chunk); derived layouts."""
    import concourse.bass as bass
    import concourse.tile as tile
    from concourse import mybir

    t_u = meta["t_u"]                       # [NB, NCHUNK] ints
    T_b = t_u.sum(axis=1)                   # tiles per block
    TILES_TOT = int(T_b.sum())
    col_base = np.concatenate([[0], np.cumsum(T_b)[:-1]]).astype(int)
    # gather stream layout: per (group, chunk): C_gc columns
    C_gc = np.zeros((NG, NCHUNK), int)
    for g in range(NG):
        blks = range(g * BPG, min((g + 1) * BPG, NB))
        for c in range(NCHUNK):
            C_gc[g, c] = sum(int(t_u[b, c]) for b in blks)
    CMAX = int(C_gc.sum(axis=1).max())
    TMAX = int(T_b.max())
    W_gc = C_gc * 8                         # idx cols per (g,c): 128*C/16
    Wtot = int(W_gc.sum())
    WMAX = int(W_gc.max())

    nc = bass.Bass(num_swdge_queues=4)
    nc.num_devices = NCORES
    f32, fp16 = mybir.dt.float32, mybir.dt.float16
    i16 = mybir.dt.int16

    xpt = nc.declare_dram_parameter("xpt", [128, NB * D_IN], fp16,
                                    isOutput=False)
    w1 = nc.declare_dram_parameter("w1", [D_IN, F], f32, isOutput=False)
    idxd = nc.declare_dram_parameter("idxd", [128, Wtot], i16, isOutput=False)
    cold = nc.declare_dram_parameter("cold", [128, TILES_TOT], i16,
                                     isOutput=False)
    dinvd = nc.declare_dram_parameter("dinvd", [128, NB], f32, isOutput=False)
    ctd = nc.declare_dram_parameter("ctd", [128, NB * 128], fp16,
                                    isOutput=False)
    b1d = nc.declare_dram_parameter("b1d", [1, F], f32, isOutput=False)
    iotad = nc.declare_dram_parameter("iotad", [1, 128], i16, isOutput=False)
    pooled = nc.declare_dram_parameter("pooled", [G, F], f32, isOutput=True)

    HALF = S // 2
    hs_loc = nc.dram_tensor("hs_loc", [S, F], fp16, kind="Internal")
    hs_shA = nc.dram_tensor("hs_shA", [NCORES * HALF, F], fp16,
                            kind="Internal", addr_space="Shared")
    hs_shB = nc.dram_tensor("hs_shB", [NCORES * HALF, F], fp16,
                            kind="Internal", addr_space="Shared")
    groups = [list(range(NCORES))]

    with tile.TileContext(nc) as tc:
        with (
            tc.tile_pool(name="meta", bufs=1) as mp_,
            tc.tile_pool(name="xin", bufs=3) as xin,
            tc.tile_pool(name="hps", bufs=2, space="PSUM") as hps,
            tc.tile_pool(name="hsb", bufs=3) as hsb,
            tc.tile_pool(name="idxp", bufs=8) as idxp,
            tc.tile_pool(name="gp", bufs=2) as gp,
            tc.tile_pool(name="mp", bufs=3) as mp,
            tc.tile_pool(name="aps", bufs=4, space="PSUM") as aps,
            tc.tile_pool(name="post", bufs=6) as post,
            tc.tile_pool(name="pps", bufs=1, space="PSUM") as pps,
        ):
            # ---- metadata loads -------------------------------------------
            col_sb = mp_.tile([128, TILES_TOT], i16)
            nc.sync.dma_start(col_sb[:, :], cold[:, :])
            dinv_sb = mp_.tile([128, NB], f32)
            nc.sync.dma_start(dinv_sb[:, :], dinvd[:, :])
            ct_sb = mp_.tile([128, NB, 128], fp16)
            nc.sync.dma_start(ct_sb[:, :, :], ctd[:, :])
            b1b = mp_.tile([128, F], f32)
            nc.sync.dma_start(b1b[:, :], b1d[:, :].to_broadcast((128, F)))
            iota_sb = mp_.tile([128, 128], i16)
            nc.sync.dma_start(iota_sb[:, :], iotad[:, :].to_broadcast((128, 128)))
            w1f = mp_.tile([128, KT, F], f32)
            for k in range(KT):
                nc.sync.dma_start(w1f[:, k, :], w1[k * 128:(k + 1) * 128, :])
            w1_sb = mp_.tile([128, KT, F], fp16)
            for k in range(KT):
                nc.scalar.activation(w1_sb[:, k, :], w1f[:, k, :],
                                     mybir.ActivationFunctionType.Copy)
            # zero-fill the pad rows of the local shard (rows RPC..S)
            zpad = mp_.tile([S - RPC, F], fp16)
            nc.vector.memset(zpad[:, :], 0.0)
            nc.sync.dma_start(hs_loc[RPC:S, :], zpad[:, :])

            def phase1(m0, m1):
                for m in range(m0, m1):
                    rows = 128 if m < NB - 1 else LAST
                    xt = xin.tile([128, KT, 128], fp16, tag="x")
                    nc.sync.dma_start(xt[:, :, :],
                                      xpt[:, m * D_IN:(m + 1) * D_IN])
                    hp = hps.tile([128, F], f32, tag="hp")
                    for k in range(KT):
                        nc.tensor.matmul(hp[:, :], xt[:, k, :], w1_sb[:, k, :],
                                         start=(k == 0), stop=(k == KT - 1))
                    hs = hsb.tile([128, F], fp16, tag="hs")
                    nc.scalar.activation(hs[:rows, :], hp[:rows, :],
                                         mybir.ActivationFunctionType.Copy,
                                         scale=dinv_sb[:rows, m:m + 1])
                    nc.sync.dma_start(hs_loc[m * 128:m * 128 + rows, :],
                                      hs[:rows, :])

            nreg_cache = {}

            def nreg(v):
                if v not in nreg_cache:
                    nreg_cache[v] = nc.gpsimd.to_reg(v)
                return nreg_cache[v]

            def phase2(pool_ps, first, last):
                ioff = 0
                qctr = [0]
                ngr = NG if ngrun is None else ngrun
                for g in range(NG):
                    if g >= ngr:
                        break
                    blks = list(range(g * BPG, min((g + 1) * BPG, NB)))
                    gt = gp.tile([128, CMAX, 128], fp16, tag="g")
                    cb = 0
                    chunk_base = {}
                    for c in range(NCHUNK):
                        chunk_base[c] = cb
                        n_gc = int(C_gc[g, c]) * 128
                        if n_gc == 0:
                            continue
                        w = int(W_gc[g, c])
                        if variant == "monly":
                            nc.vector.memset(gt[:, cb:cb + C_gc[g, c], :], 0.0)
                        elif variant == "sp1024":
                            idxt = idxp.tile([128, WMAX], i16, tag="i")
                            nc.sync.dma_start(idxt[:, :w],
                                              idxd[:, ioff:ioff + w])
                            cc = int(C_gc[g, c])
                            srcq = (hs_shA[c * CH:(c + 1) * CH, :] if c < 2
                                    else hs_shB[(c - 2) * CH:(c - 1) * CH, :])
                            for k0 in range(0, cc, 8):
                                kc = min(8, cc - k0)
                                nk = kc * 128
                                nc.gpsimd.dma_gather(
                                    gt[:, cb + k0:cb + k0 + kc, :],
                                    srcq,
                                    idxt[:, k0 * 8:k0 * 8 + kc * 8],
                                    nk, nreg(nk), 128,
                                    elem_step=128, single_packet=True,
                                    queue_num=qctr[0] % 4)
                                qctr[0] += 1
                        else:
                            idxt = idxp.tile([128, WMAX], i16, tag="i")
                            nc.sync.dma_start(idxt[:, :w],
                                              idxd[:, ioff:ioff + w])
                            src = (hs_shA[c * CH:(c + 1) * CH, :] if c < 2
                                   else hs_shB[(c - 2) * CH:(c - 1) * CH, :])
                            nc.gpsimd.dma_gather(
                                gt[:, cb:cb + C_gc[g, c], :],
                                src,
                                idxt[:, :w], n_gc, nreg(n_gc), 128,
                                elem_step=128, single_packet=False,
                                queue_num=c % 4)
                        ioff += w
                        cb += int(C_gc[g, c])
                    if variant == "gonly":
                        continue
                    for b in blks:
                        tb = int(T_b[b])
                        mt = mp.tile([128, TMAX, 128], fp16, tag="m")
                        nc.vector.tensor_tensor(
                            out=mt[:, :tb, :],
                            in0=col_sb[:, col_base[b]:col_base[b] + tb]
                                .unsqueeze(2).to_broadcast((128, tb, 128)),
                            in1=iota_sb[:, :].unsqueeze(1)
                                .to_broadcast((128, tb, 128)),
                            op=mybir.AluOpType.is_equal)
                        ap = aps.tile([128, F], f32, tag="agg")
                        ki = 0
                        for c in range(NCHUNK):
                            # this block's tiles within the (g,c) segment
                            pre = sum(int(t_u[b2, c]) for b2 in blks
                                      if b2 < b)
                            for k in range(int(t_u[b, c])):
                                gcol = chunk_base[c] + pre + k
                                nc.tensor.matmul(
                                    ap[:, :], mt[:, ki, :], gt[:, gcol, :],
                                    start=(ki == 0), stop=(ki == tb - 1))
                                ki += 1
                        s1 = post.tile([128, F], f32, tag="s1")
                        nc.scalar.activation(s1[:, :], ap[:, :],
                                             mybir.ActivationFunctionType.Copy,
                                             scale=dinv_sb[:, b:b + 1])
                        s2 = post.tile([128, F], f32, tag="s2")
                        nc.vector.tensor_tensor(out=s2[:, :], in0=s1[:, :],
                                                in1=b1b[:, :],
                                                op=mybir.AluOpType.add)
                        h1 = post.tile([128, F], fp16, tag="h1")
                        nc.scalar.activation(h1[:, :], s2[:, :],
                                             mybir.ActivationFunctionType.Relu)
                        nc.tensor.matmul(pool_ps[:, :], ct_sb[:, b, :],
                                         h1[:, :],
                                         start=(first and b == 0),
                                         stop=(last and b == NB - 1))

            pool_ps = pps.tile([G, F], f32)
            for _rep in range(reps):
                if phases >= 1:
                    phase1(0, NB // 2)
                if phases >= 2:
                    nc.gpsimd.collective_compute(
                        "AllGather", mybir.AluOpType.bypass,
                        replica_groups=groups,
                        ins=[hs_loc[0:HALF, :]], outs=[hs_shA[:, :]])
                if phases >= 1:
                    phase1(NB // 2, NB)
                if phases >= 2:
                    nc.gpsimd.collective_compute(
                        "AllGather", mybir.AluOpType.bypass,
                        replica_groups=groups,
                        ins=[hs_loc[HALF:S, :]], outs=[hs_shB[:, :]])
                if phases >= 3:
                    phase2(pool_ps, _rep == 0, _rep == reps - 1)

            pout = post.tile([G, F], f32, tag="po")
            if phases >= 3 and variant not in ("gonly",):
                nc.vector.tensor_copy(pout[:, :], pool_ps[:, :])
            else:
                nc.vector.memset(pout[:, :], 0.0)
            nc.sync.dma_start(pooled[:, :], pout[:, :])

    return _finish_build(nc)


def preprocess(x, edge_index, batch):
    """Bucket edges by (core, dest block, src chunk); build padded per-core
    device arrays + the uniform tile structure."""
    ei = np.asarray(edge_index)
    row = np.concatenate([ei[0].astype(np.int64), np.arange(N, dtype=np.int64)])
    col = np.concatenate([ei[1].astype(np.int64), np.arange(N, dtype=np.int64)])
    ne = row.shape[0]

    deg = np.bincount(col, minlength=N).astype(np.float32)
    dinv = 1.0 / np.sqrt(deg)

    core = col // RPC
    loc = col - core * RPC
    blk = loc // 128
    d_in_blk = (loc % 128).astype(np.int16)
    HALF = S // 2
    _q = row // RPC
    _r = row % RPC
    src_g = np.where(_r < HALF, _q * HALF + _r,
                     NCORES * HALF + _q * HALF + (_r - HALF))
    chunk = src_g // CH
    src_loc = (src_g - chunk * CH).astype(np.int16)

    # bucket id: ((core*NB + blk)*NCHUNK + chunk)
    bkt = ((core * NB + blk) * NCHUNK + chunk).astype(np.int64)
    order = np.argsort(bkt, kind="stable")
    nbkt = NCORES * NB * NCHUNK
    cnt = np.bincount(bkt, minlength=nbkt)
    starts = np.zeros(nbkt + 1, np.int64)
    np.cumsum(cnt, out=starts[1:])
    q = np.arange(ne, dtype=np.int64) - starts[bkt[order]]  # pos within bucket

    cnt4 = cnt.reshape(NCORES, NB, NCHUNK)
    t_u = np.ceil(cnt4.max(axis=0) / 128).astype(np.int64)  # [NB, NCHUNK]
    T_b = t_u.sum(axis=1)
    TILES_TOT = int(T_b.sum())
    col_base = np.concatenate([[0], np.cumsum(T_b)[:-1]]).astype(np.int64)

    # per-(g,c) gather stream offsets
    C_gc = np.zeros((NG, NCHUNK), np.int64)
    for g in range(NG):
        blks = range(g * BPG, min((g + 1) * BPG, NB))
        for c in range(NCHUNK):
            C_gc[g, c] = sum(int(t_u[b, c]) for b in blks)
    W_gc = C_gc * 8
    Wtot = int(W_gc.sum())

    # stream base (in slots) for each (g, c) and block-within-group offsets
    stream_base = np.zeros((NG, NCHUNK), np.int64)  # slot offset of (g,c)
    acc = 0
    for g in range(NG):
        for c in range(NCHUNK):
            stream_base[g, c] = acc
            acc += int(C_gc[g, c]) * 128
    tot_slots = acc
    # slot offset of bucket (b, c) inside its (g, c) stream
    bkt_slot_base = np.zeros((NB, NCHUNK), np.int64)
    for g in range(NG):
        blks = list(range(g * BPG, min((g + 1) * BPG, NB)))
        for c in range(NCHUNK):
            off = stream_base[g, c]
            for b in blks:
                bkt_slot_base[b, c] = off
                off += int(t_u[b, c]) * 128

    bt = np.asarray(batch).astype(np.int64)
    # c[g, j] = dinv[j] * sum over edges j->i with batch[i]=g of dinv[i]
    cmat = np.zeros((G, N), np.float32)
    np.add.at(cmat, (bt[col], row), dinv[col])
    cmat *= dinv[None, :]

    x = np.asarray(x, np.float32)
    in_maps = []
    ord_core = core[order]
    ord_bc = (blk[order], chunk[order])
    ord_slot = (bkt_slot_base[ord_bc[0], ord_bc[1]] + q)
    ord_src = src_loc[order]
    ord_d = d_in_blk[order]
    ord_blk = ord_bc[0]
    ord_chunk = ord_bc[1]
    # per-block tile offset of chunk c: prefix sum of t_u over chunks
    t_pre = np.zeros((NB, NCHUNK), np.int64)
    t_pre[:, 1:] = np.cumsum(t_u, axis=1)[:, :-1]

    for i in range(NCORES):
        sel = ord_core == i
        slots = ord_slot[sel]
        # gather index stream
        L = np.zeros(tot_slots, np.int16)  # pad (killed by col=-1)
        L[slots] = ord_src[sel]
        idx16 = L.reshape(-1, 16).T.astype(np.int16)   # [16, tot/16]
        idxd = np.tile(idx16, (8, 1))
        # col array (block-contiguous tile order)
        carr = np.full((128, TILES_TOT), -1, np.int16)
        bsel = ord_blk[sel]
        csel = ord_chunk[sel]
        qsel = slots - bkt_slot_base[bsel, csel]
        tile_idx = col_base[bsel] + t_pre[bsel, csel] + qsel // 128
        carr[qsel % 128, tile_idx] = ord_d[sel]

        xs = x[i * RPC:(i + 1) * RPC]
        xs = np.concatenate([xs, np.zeros((S - RPC, D_IN), np.float32)])
        # xpt[p, m*768 + k*128 + cc] = x[m*128+cc, k*128+p]
        xr = xs.reshape(NB, 128, KT, 128).transpose(3, 0, 2, 1)  # p? see below
        # xr[pp, m, k, cc] = xs[m*128+cc, k*128+pp]
        xpt = np.ascontiguousarray(
            xr.reshape(128, NB * D_IN)).astype(np.float16)

        dinv_pad = np.zeros(S, np.float32)
        dinv_pad[:RPC] = dinv[i * RPC:(i + 1) * RPC]
        dinvd = np.ascontiguousarray(
            dinv_pad.reshape(NB, 128).T)

        cslice = np.zeros((G, S), np.float32)
        cslice[:, :RPC] = cmat[:, i * RPC:(i + 1) * RPC]
        # ctd[p, m*128 + g] = c[g, m*128+p]
        ctd = np.ascontiguousarray(
            cslice.reshape(G, NB, 128).transpose(2, 1, 0)
            .reshape(128, NB * 128)).astype(np.float16)

        in_maps.append({
            "xpt": xpt,
            "idxd": idxd,
            "cold": carr,
            "dinvd": dinvd,
            "ctd": ctd,
            "iotad": np.arange(128, dtype=np.int16).reshape(1, 128),
        })
    meta = {"t_u": t_u}
    return in_maps, meta, dinv


def finish(pooled_sum, batch, W2, b2):
    cnts = np.maximum(np.bincount(np.asarray(batch).astype(np.int64),
                                  minlength=G).astype(np.float32), 1.0)
    logits = (pooled_sum @ np.asarray(W2, np.float32)) / cnts[:, None] \
        + np.asarray(b2, np.float32)
    m = logits.max(axis=1, keepdims=True)
    ls = m + np.log(np.exp(logits - m).sum(axis=1, keepdims=True))
    return (logits - ls).astype(np.float32)


def kernel(x, edge_index, batch, W1, b1, W2, b2):
    from concourse.bass_utils import run_bass_kernel_spmd

    in_maps, meta, _ = preprocess(x, edge_index, batch)
    W1 = np.asarray(W1, np.float32)
    b1 = np.asarray(b1, np.float32).reshape(1, F)
    for m in in_maps:
        m["w1"] = W1
        m["b1d"] = b1
    nc = build_nc(meta)
    res = run_bass_kernel_spmd(nc, in_maps, list(range(NCORES))).results
    pooled_sum = np.zeros((G, F), np.float32)
    for i in range(NCORES):
        pooled_sum += np.asarray(res[i]["pooled"], np.float32)
    return finish(pooled_sum, batch, W2, b2)


# revision 35
# speedup vs baseline: 1.0239x; 1.0132x over previous
"""GCN classifier on 8 trn2 NeuronCores.

Sharding: nodes (and their incoming edges) are partitioned contiguously
across the 8 cores.  Each core computes its slice of h = dinv * (x @ W1)
(x is host-transposed to fp16 so no on-device transposes), the slices are
AllGathered into a shared fp16 node table, and each core aggregates its
destination nodes' edges by gathering source rows with batched dma_gather
(SWDGE ucode, ~5K rows per instruction, int16-indexed so the table is
split into 4 chunks; alternating over 4 SWDGE queues hides descriptor-ring
drain backpressure) and reducing 128-edge tiles onto 128-destination
blocks with one-hot selection matmuls (PE).  The second GCN layer + mean
pool are algebraically folded: pooled_sum[g] = sum_j c[g,j] * h1[j] with
c[g,j] = dinv[j] * sum_{i in g} A[i,j] dinv[i] precomputed on the host,
so layer 2 costs one [128x128x128] matmul per block and the second
gather/AllGather disappears.  W2/b2/log_softmax are applied to the tiny
[128,128] pooled partials on the host.
"""
import numpy as np

N = 100000
G = 128
NCORES = 8
RPC = N // NCORES        # real rows per core (12500)
NB = 98                  # blocks per core: ceil(12500/128)
S = NB * 128             # padded rows per core shard (12544)
D_IN, F = 768, 128
KT = D_IN // 128
LAST = RPC - 97 * 128    # rows in last block (84)
NCHUNK = 4
CH = 2 * S               # chunk rows (25088 < 32768, int16-addressable)
BPG = 4                  # dest blocks per gather group
NG = (NB + BPG - 1) // BPG


# --------------------------------------------------------------------------
# walrus in this environment rejects >1 sync wait per instruction; split
# extras onto inserted drains (semantically identical, engine blocks on each)
def _split_waits(nc, maxw=1):
    from concourse import mybir
    for fn in nc.m.functions:
        for bb in fn.blocks:
            newlist = []
            for inst in bb.instructions:
                si = getattr(inst, "sync_info", None)
                if si is not None and len(si.on_wait) > maxw:
                    waits = list(si.on_wait)
                    chunks = [waits[i:i + maxw] for i in range(0, len(waits), maxw)]
                    for ci, ch in enumerate(chunks[:-1]):
                        newlist.append(mybir.InstDrain(
                            name=f"{inst.name}-wsplit{ci}", ins=[], outs=[],
                            engine=inst.engine,
                            sync_info=mybir.SyncInfo(on_wait=list(ch), on_update=[]),
                        ))
                    si.on_wait = list(chunks[-1])
                newlist.append(inst)
            bb.instructions = newlist
    return nc


def _finish_build(nc):
    """Insert GPSIMD ucode library loads (dma_gather lives in the 'mlp'
    library) and lower the pseudo reload instructions to pool ISA, then
    split multi-wait instructions for this walrus."""
    import bass_rust
    from concourse import mybir
    from concourse import library_config as lc
    mask = {}
    for lib in lc.all_libraries:
        for t in lib.instructions:
            mask[t] = mask.get(t, 0) | (1 << lib.index)
    bass_rust.insert_library_loads(nc, mask, len(lc.all_libraries),
                                   lc.standard.index)
    mybir.codegen_inst_isa_subclasses(nc)
    return _split_waits(nc)


def build_nc(meta, phases=3, reps=1, variant="full", ngrun=None):
    """meta: dict with uniform (cross-core) tile structure:
    t_u [NB, NCHUNK] tiles per (block, chunk); derived layouts."""
    import concourse.bass as bass
    import concourse.tile as tile
    from concourse import mybir

    t_u = meta["t_u"]                       # [NB, NCHUNK] ints
    T_b = t_u.sum(axis=1)                   # tiles per block
    TILES_TOT = int(T_b.sum())
    col_base = np.concatenate([[0], np.cumsum(T_b)[:-1]]).astype(int)
    # gather stream layout: per (group, chunk): C_gc columns
    C_gc = np.zeros((NG, NCHUNK), int)
    for g in range(NG):
        blks = range(g * BPG, min((g + 1) * BPG, NB))
        for c in range(NCHUNK):
            C_gc[g, c] = sum(int(t_u[b, c]) for b in blks)
    CMAX = int(C_gc.sum(axis=1).max())
    TMAX = int(T_b.max())
    W_gc = C_gc * 8                         # idx cols per (g,c): 128*C/16
    Wtot = int(W_gc.sum())
    WMAX = int(W_gc.max())

    nc = bass.Bass(num_swdge_queues=4)
    nc.num_devices = NCORES
    f32, fp16 = mybir.dt.float32, mybir.dt.float16
    i16 = mybir.dt.int16

    xpt = nc.declare_dram_parameter("xpt", [128, NB * D_IN], fp16,
                                    isOutput=False)
    w1 = nc.declare_dram_parameter("w1", [D_IN, F], f32, isOutput=False)
    idxd = nc.declare_dram_parameter("idxd", [128, Wtot], i16, isOutput=False)
    cold = nc.declare_dram_parameter("cold", [128, TILES_TOT], i16,
                                     isOutput=False)
    dinvd = nc.declare_dram_parameter("dinvd", [128, NB], f32, isOutput=False)
    ctd = nc.declare_dram_parameter("ctd", [128, NB * 128], fp16,
                                    isOutput=False)
    b1d = nc.declare_dram_parameter("b1d", [1, F], f32, isOutput=False)
    iotad = nc.declare_dram_parameter("iotad", [1, 128], i16, isOutput=False)
    pooled = nc.declare_dram_parameter("pooled", [G, F], f32, isOutput=True)

    HALF = S // 2
    hs_loc = nc.dram_tensor("hs_loc", [S, F], fp16, kind="Internal")
    hs_shA = nc.dram_tensor("hs_shA", [NCORES * HALF, F], fp16,
                            kind="Internal", addr_space="Shared")
    hs_shB = nc.dram_tensor("hs_shB", [NCORES * HALF, F], fp16,
                            kind="Internal", addr_space="Shared")
    groups = [list(range(NCORES))]

    with tile.TileContext(nc) as tc:
        with (
            tc.tile_pool(name="meta", bufs=1) as mp_,
            tc.tile_pool(name="xin", bufs=3) as xin,
            tc.tile_pool(name="hps", bufs=2, space="PSUM") as hps,
            tc.tile_pool(name="hsb", bufs=3) as hsb,
            tc.tile_pool(name="idxp", bufs=8) as idxp,
            tc.tile_pool(name="gp", bufs=2) as gp,
            tc.tile_pool(name="mp", bufs=3) as mp,
            tc.tile_pool(name="aps", bufs=4, space="PSUM") as aps,
            tc.tile_pool(name="post", bufs=6) as post,
            tc.tile_pool(name="pps", bufs=1, space="PSUM") as pps,
        ):
            # ---- metadata loads -------------------------------------------
            col_sb = mp_.tile([128, TILES_TOT], i16)
            nc.sync.dma_start(col_sb[:, :], cold[:, :])
            dinv_sb = mp_.tile([128, NB], f32)
            nc.sync.dma_start(dinv_sb[:, :], dinvd[:, :])
            ct_sb = mp_.tile([128, NB, 128], fp16)
            nc.sync.dma_start(ct_sb[:, :, :], ctd[:, :])
            b1b = mp_.tile([128, F], f32)
            nc.sync.dma_start(b1b[:, :], b1d[:, :].to_broadcast((128, F)))
            iota_sb = mp_.tile([128, 128], i16)
            nc.sync.dma_start(iota_sb[:, :], iotad[:, :].to_broadcast((128, 128)))
            w1f = mp_.tile([128, KT, F], f32)
            for k in range(KT):
                nc.sync.dma_start(w1f[:, k, :], w1[k * 128:(k + 1) * 128, :])
            w1_sb = mp_.tile([128, KT, F], fp16)
            for k in range(KT):
                nc.scalar.activation(w1_sb[:, k, :], w1f[:, k, :],
                                     mybir.ActivationFunctionType.Copy)
            # zero-fill the pad rows of the local shard (rows RPC..S)
            zpad = mp_.tile([S - RPC, F], fp16)
            nc.vector.memset(zpad[:, :], 0.0)
            nc.sync.dma_start(hs_loc[RPC:S, :], zpad[:, :])

            def phase1(m0, m1):
                for m in range(m0, m1):
                    rows = 128 if m < NB - 1 else LAST
                    xt = xin.tile([128, KT, 128], fp16, tag="x")
                    nc.sync.dma_start(xt[:, :, :],
                                      xpt[:, m * D_IN:(m + 1) * D_IN])
                    hp = hps.tile([128, F], f32, tag="hp")
                    for k in range(KT):
                        nc.tensor.matmul(hp[:, :], xt[:, k, :], w1_sb[:, k, :],
                                         start=(k == 0), stop=(k == KT - 1))
                    hs = hsb.tile([128, F], fp16, tag="hs")
                    nc.scalar.activation(hs[:rows, :], hp[:rows, :],
                                         mybir.ActivationFunctionType.Copy,
                                         scale=dinv_sb[:rows, m:m + 1])
                    nc.sync.dma_start(hs_loc[m * 128:m * 128 + rows, :],
                                      hs[:rows, :])

            nreg_cache = {}

            def nreg(v):
                if v not in nreg_cache:
                    nreg_cache[v] = nc.gpsimd.to_reg(v)
                return nreg_cache[v]

            def phase2(pool_ps, first, last):
                ioff = 0
                qctr = [0]
                ngr = NG if ngrun is None else ngrun
                for g in range(NG):
                    if g >= ngr:
                        break
                    blks = list(range(g * BPG, min((g + 1) * BPG, NB)))
                    gt = gp.tile([128, CMAX, 128], fp16, tag="g")
                    cb = 0
                    chunk_base = {}
                    for c in range(NCHUNK):
                        chunk_base[c] = cb
                        n_gc = int(C_gc[g, c]) * 128
                        if n_gc == 0:
                            continue
                        w = int(W_gc[g, c])
                        if variant == "monly":
                            nc.vector.memset(gt[:, cb:cb + C_gc[g, c], :], 0.0)
                        elif variant == "sp1024":
                            idxt = idxp.tile([128, WMAX], i16, tag="i")
                            nc.sync.dma_start(idxt[:, :w],
                                              idxd[:, ioff:ioff + w])
                            cc = int(C_gc[g, c])
                            srcq = (hs_shA[c * CH:(c + 1) * CH, :] if c < 2
                                    else hs_shB[(c - 2) * CH:(c - 1) * CH, :])
                            for k0 in range(0, cc, 8):
                                kc = min(8, cc - k0)
                                nk = kc * 128
                                nc.gpsimd.dma_gather(
                                    gt[:, cb + k0:cb + k0 + kc, :],
                                    srcq,
                                    idxt[:, k0 * 8:k0 * 8 + kc * 8],
                                    nk, nreg(nk), 128,
                                    elem_step=128, single_packet=True,
                                    queue_num=qctr[0] % 4)
                                qctr[0] += 1
                        else:
                            idxt = idxp.tile([128, WMAX], i16, tag="i")
                            nc.sync.dma_start(idxt[:, :w],
                                              idxd[:, ioff:ioff + w])
                            src = (hs_shA[c * CH:(c + 1) * CH, :] if c < 2
                                   else hs_shB[(c - 2) * CH:(c - 1) * CH, :])
                            nc.gpsimd.dma_gather(
                                gt[:, cb:cb + C_gc[g, c], :],
                                src,
                                idxt[:, :w], n_gc, nreg(n_gc), 128,
                                elem_step=128, single_packet=False,
                                queue_num=c % 4)
                        ioff += w
                        cb += int(C_gc[g, c])
                    if variant == "gonly":
                        continue
                    for b in blks:
                        tb = int(T_b[b])
                        mt = mp.tile([128, TMAX, 128], fp16, tag="m")
                        nc.vector.tensor_tensor(
                            out=mt[:, :tb, :],
                            in0=col_sb[:, col_base[b]:col_base[b] + tb]
                                .unsqueeze(2).to_broadcast((128, tb, 128)),
                            in1=iota_sb[:, :].unsqueeze(1)
                                .to_broadcast((128, tb, 128)),
                            op=mybir.AluOpType.is_equal)
                        ap = aps.tile([128, F], f32, tag="agg")
                        ki = 0
                        for c in range(NCHUNK):
                            # this block's tiles within the (g,c) segment
                            pre = sum(int(t_u[b2, c]) for b2 in blks
                                      if b2 < b)
                            for k in range(int(t_u[b, c])):
                                gcol = chunk_base[c] + pre + k
                                nc.tensor.matmul(
                                    ap[:, :], mt[:, ki, :], gt[:, gcol, :],
                                    start=(ki == 0), stop=(ki == tb - 1))
                                ki += 1
                        s1 = post.tile([128, F], f32, tag="s1")
                        nc.scalar.activation(s1[:, :], ap[:, :],
                                             mybir.ActivationFunctionType.Copy,
                                             scale=dinv_sb[:, b:b + 1])
                        s2 = post.tile([128, F], f32, tag="s2")
                        nc.vector.tensor_tensor(out=s2[:, :], in0=s1[:, :],
                                                in1=b1b[:, :],
                                                op=mybir.AluOpType.add)
                        h1 = post.tile([128, F], fp16, tag="h1")
                        nc.scalar.activation(h1[:, :], s2[:, :],
                                             mybir.ActivationFunctionType.Relu)
                        nc.tensor.matmul(pool_ps[:, :], ct_sb[:, b, :],
                                         h1[:, :],
                                         start=(first and b == 0),
                                         stop=(last and b == NB - 1))

            pool_ps = pps.tile([G, F], f32)
            for _rep in range(reps):
                if phases >= 1:
                    phase1(0, NB // 2)
                if phases >= 2:
                    nc.gpsimd.collective_compute(
                        "AllGather", mybir.AluOpType.bypass,
                        replica_groups=groups,
                        ins=[hs_loc[0:HALF, :]], outs=[hs_shA[:, :]])
                if phases >= 1:
                    phase1(NB // 2, NB)
                if phases >= 2:
                    nc.gpsimd.collective_compute(
                        "AllGather", mybir.AluOpType.bypass,
                        replica_groups=groups,
                        ins=[hs_loc[HALF:S, :]], outs=[hs_shB[:, :]])
                if phases >= 3:
                    phase2(pool_ps, _rep == 0, _rep == reps - 1)

            pout = post.tile([G, F], f32, tag="po")
            if phases >= 3 and variant not in ("gonly",):
                nc.vector.tensor_copy(pout[:, :], pool_ps[:, :])
            else:
                nc.vector.memset(pout[:, :], 0.0)
            nc.sync.dma_start(pooled[:, :], pout[:, :])

    return _finish_build(nc)


def preprocess(x, edge_index, batch):
    """Bucket edges by (core, dest block, src chunk); build padded per-core
    device arrays + the uniform tile structure."""
    ei = np.asarray(edge_index)
    row = np.concatenate([ei[0].astype(np.int64), np.arange(N, dtype=np.int64)])
    col = np.concatenate([ei[1].astype(np.int64), np.arange(N, dtype=np.int64)])
    ne = row.shape[0]

    deg = np.bincount(col, minlength=N).astype(np.float32)
    dinv = 1.0 / np.sqrt(deg)

    core = col // RPC
    loc = col - core * RPC
    blk = loc // 128
    d_in_blk = (loc % 128).astype(np.int16)
    HALF = S // 2
    _q = row // RPC
    _r = row % RPC
    src_g = np.where(_r < HALF, _q * HALF + _r,
                     NCORES * HALF + _q * HALF + (_r - HALF))
    chunk = src_g // CH
    src_loc = (src_g - chunk * CH).astype(np.int16)

    # bucket id: ((core*NB + blk)*NCHUNK + chunk)
    bkt = ((core * NB + blk) * NCHUNK + chunk).astype(np.int64)
    order = np.argsort(bkt, kind="stable")
    nbkt = NCORES * NB * NCHUNK
    cnt = np.bincount(bkt, minlength=nbkt)
    starts = np.zeros(nbkt + 1, np.int64)
    np.cumsum(cnt, out=starts[1:])
    q = np.arange(ne, dtype=np.int64) - starts[bkt[order]]  # pos within bucket

    cnt4 = cnt.reshape(NCORES, NB, NCHUNK)
    t_u = np.ceil(cnt4.max(axis=0) / 128).astype(np.int64)  # [NB, NCHUNK]
    T_b = t_u.sum(axis=1)
    TILES_TOT = int(T_b.sum())
    col_base = np.concatenate([[0], np.cumsum(T_b)[:-1]]).astype(np.int64)

    # per-(g,c) gather stream offsets
    C_gc = np.zeros((NG, NCHUNK), np.int64)
    for g in range(NG):
        blks = range(g * BPG, min((g + 1) * BPG, NB))
        for c in range(NCHUNK):
            C_gc[g, c] = sum(int(t_u[b, c]) for b in blks)
    W_gc = C_gc * 8
    Wtot = int(W_gc.sum())

    # stream base (in slots) for each (g, c) and block-within-group offsets
    stream_base = np.zeros((NG, NCHUNK), np.int64)  # slot offset of (g,c)
    acc = 0
    for g in range(NG):
        for c in range(NCHUNK):
            stream_base[g, c] = acc
            acc += int(C_gc[g, c]) * 128
    tot_slots = acc
    # slot offset of bucket (b, c) inside its (g, c) stream
    bkt_slot_base = np.zeros((NB, NCHUNK), np.int64)
    for g in range(NG):
        blks = list(range(g * BPG, min((g + 1) * BPG, NB)))
        for c in range(NCHUNK):
            off = stream_base[g, c]
            for b in blks:
                bkt_slot_base[b, c] = off
                off += int(t_u[b, c]) * 128

    bt = np.asarray(batch).astype(np.int64)
    # c[g, j] = dinv[j] * sum over edges j->i with batch[i]=g of dinv[i]
    cmat = np.zeros((G, N), np.float32)
    np.add.at(cmat, (bt[col], row), dinv[col])
    cmat *= dinv[None, :]

    x = np.asarray(x, np.float32)
    in_maps = []
    ord_core = core[order]
    ord_bc = (blk[order], chunk[order])
    ord_slot = (bkt_slot_base[ord_bc[0], ord_bc[1]] + q)
    ord_src = src_loc[order]
    ord_d = d_in_blk[order]
    ord_blk = ord_bc[0]
    ord_chunk = ord_bc[1]
    # per-block tile offset of chunk c: prefix sum of t_u over chunks
    t_pre = np.zeros((NB, NCHUNK), np.int64)
    t_pre[:, 1:] = np.cumsum(t_u, axis=1)[:, :-1]

    for i in range(NCORES):
        sel = ord_core == i
        slots = ord_slot[sel]
        # gather index stream
        L = np.zeros(tot_slots, np.int16)  # pad (killed by col=-1)
        L[slots] = ord_src[sel]
        idx16 = L.reshape(-1, 16).T.astype(np.int16)   # [16, tot/16]
        idxd = np.tile(idx16, (8, 1))
        # col array (block-contiguous tile order)
        carr = np.full((128, TILES_TOT), -1, np.int16)
        bsel = ord_blk[sel]
        csel = ord_chunk[sel]
        qsel = slots - bkt_slot_base[bsel, csel]
        tile_idx = col_base[bsel] + t_pre[bsel, csel] + qsel // 128
        carr[qsel % 128, tile_idx] = ord_d[sel]

        xs = x[i * RPC:(i + 1) * RPC]
        xs = np.concatenate([xs, np.zeros((S - RPC, D_IN), np.float32)])
        # xpt[p, m*768 + k*128 + cc] = x[m*128+cc, k*128+p]
        xr = xs.reshape(NB, 128, KT, 128).transpose(3, 0, 2, 1)  # p? see below
        # xr[pp, m, k, cc] = xs[m*128+cc, k*128+pp]
        xpt = np.ascontiguousarray(
            xr.reshape(128, NB * D_IN)).astype(np.float16)

        dinv_pad = np.zeros(S, np.float32)
        dinv_pad[:RPC] = dinv[i * RPC:(i + 1) * RPC]
        dinvd = np.ascontiguousarray(
            dinv_pad.reshape(NB, 128).T)

        cslice = np.zeros((G, S), np.float32)
        cslice[:, :RPC] = cmat[:, i * RPC:(i + 1) * RPC]
        # ctd[p, m*128 + g] = c[g, m*128+p]
        ctd = np.ascontiguousarray(
            cslice.reshape(G, NB, 128).transpose(2, 1, 0)
            .reshape(128, NB * 128)).astype(np.float16)

        in_maps.append({
            "xpt": xpt,
            "idxd": idxd,
            "cold": carr,
            "dinvd": dinvd,
            "ctd": ctd,
            "iotad": np.arange(128, dtype=np.int16).reshape(1, 128),
        })
    meta = {"t_u": t_u}
    return in_maps, meta, dinv


def finish(pooled_sum, batch, W2, b2):
    cnts = np.maximum(np.bincount(np.asarray(batch).astype(np.int64),
                                  minlength=G).astype(np.float32), 1.0)
    logits = (pooled_sum @ np.asarray(W2, np.float32)) / cnts[:, None] \
        + np.asarray(b2, np.float32)
    m = logits.max(axis=1, keepdims=True)
    ls = m + np.log(np.exp(logits - m).sum(axis=1, keepdims=True))
    return (logits - ls).astype(np.float32)


def kernel(x, edge_index, batch, W1, b1, W2, b2):
    from concourse.bass_utils import run_bass_kernel_spmd

    in_maps, meta, _ = preprocess(x, edge_index, batch)
    W1 = np.asarray(W1, np.float32)
    b1 = np.asarray(b1, np.float32).reshape(1, F)
    for m in in_maps:
        m["w1"] = W1
        m["b1d"] = b1
    nc = build_nc(meta)
    res = run_bass_kernel_spmd(nc, in_maps, list(range(NCORES))).results
    pooled_sum = np.zeros((G, F), np.float32)
    for i in range(NCORES):
        pooled_sum += np.asarray(res[i]["pooled"], np.float32)
    return finish(pooled_sum, batch, W2, b2)
